# revision 21
# baseline (speedup 1.0000x reference)
# Multi-head attention with RoPE, tensor-parallel over (head-quad, batch) on
# 8 NeuronCores.
#
# Problem: B=2, N=2048, D=1024, H=16 heads, head_dim=64.
#   q/k/v = x @ W{q,k,v}.T + b;  RoPE(q), RoPE(k);  softmax(q k^T / 8) v;
#   out = attn @ Wo.T + bo.
#
# Sharding: core c handles batch b = c // 4 and heads 4g..4g+3 (g = c % 4),
# i.e. 4 heads x 1 batch: same compute per core as 2 heads x 2 batches, but
# half the x input DMA and half the y output DMA. Each core emits a partial
# [N, D] output (bf16) for its batch; host sums 4 partials per batch + bo.
#
# Per-core layout (all matmul inputs bf16, fp32 accumulation):
#   xT   [D, N]      : batch slice of x, transposed on host.
#   q,k  [128, hp, N]: "transposed" activations; hp = head pair (2x2 heads).
#                      Biases folded via tensor_scalar during PSUM->SBUF evac.
#   RoPE             : rotate-half via partition-shifted DVE muls (sign folded
#                      into the sin table) - no PE permutation matmul.
#   v                : projected as vT (tokens moving, N=512/matmul), bias
#                      folded on evac, PE-transposed per 128-token chunk into
#                      v_sb [tk, chunk, head4, 64+1] = [v_h | 1].
#   scores^T         : [tk, tq] via lhsT=k chunk, rhs=q (free dim 512).
#   attn@v           : oT[65, tq] = [v_h|1]^T @ expT accumulated over tk in
#                      PSUM; rows 0..63 = out, row 64 = softmax denominator.
#   normalize        : reciprocal_approx_fast on the denom row, f32r round,
#                      K=1 ones-matmul broadcast to 128 partitions, multiplied
#                      into stacked on_sb [128, hp, tq] on DVE.
#   out-proj         : y[t,:] = sum_hp on_hp^T @ woT_hp (K=128 per hp).
#                      bo is added by the host after the partial-sum gather.

import numpy as np
import ml_dtypes

import concourse.bass as bass
import concourse.mybir as mybir
import concourse.tile as tile
from concourse import bacc

B, N, D, H = 2, 2048, 1024, 16
HD = 64
T = B * N
NCORES = 8
HPC = 2                   # heads per pair
NHP = 2                   # head pairs per core (4 heads)
E = HPC * HD              # 128 columns per head pair
KD = D // 128             # 8 contraction tiles for d
ROPE_BASE = 10000.0

BF = mybir.dt.bfloat16
F32 = mybir.dt.float32
F32R = mybir.dt.float32r

TQC = 1024                # tq chunk (exp granularity / psum width)
NTQC = N // TQC           # 2
NKC = N // 128            # 16 key chunks
NSL = N // 512            # 4 token slices for projections


def build_nc():
    nc = bacc.Bacc(trn_type="TRN2", target_bir_lowering=False, debug=False)

    xT = nc.dram_tensor("xT", [D, N], BF, kind="ExternalInput").ap()
    wqT = nc.dram_tensor("wqT", [D, NHP, E], BF, kind="ExternalInput").ap()
    wkT = nc.dram_tensor("wkT", [D, NHP, E], BF, kind="ExternalInput").ap()
    wvT = nc.dram_tensor("wvT", [D, NHP, E], BF, kind="ExternalInput").ap()
    woT = nc.dram_tensor("woT", [E, NHP, D], BF, kind="ExternalInput").ap()
    bcol = nc.dram_tensor("bcol", [E, NHP, 3], F32, kind="ExternalInput").ap()
    cosb = nc.dram_tensor("cosb", [E, N], BF, kind="ExternalInput").ap()
    sinb = nc.dram_tensor("sinb", [E, N], BF, kind="ExternalInput").ap()
    rotT = nc.dram_tensor("rotT", [E, E], BF, kind="ExternalInput").ap()
    idT = nc.dram_tensor("idT", [E, E], BF, kind="ExternalInput").ap()
    ones1 = nc.dram_tensor("ones1", [1, TQC], F32R, kind="ExternalInput").ap()
    y = nc.dram_tensor("y", [N, D], BF, kind="ExternalOutput").ap()

    with tile.TileContext(nc) as tc:
        _build(tc, nc, xT, wqT, wkT, wvT, woT, bcol, cosb, sinb, rotT, idT,
               ones1, y)
    nc.compile()
    return nc


def _build(tc, nc, xT, wqT, wkT, wvT, woT, bcol, cosb, sinb, rotT, idT,
           ones1, y):
    with (
        tc.tile_pool(name="consts", bufs=1) as consts,
        tc.tile_pool(name="xbig", bufs=1) as xbig,
        tc.tile_pool(name="acts", bufs=1) as acts,
        tc.tile_pool(name="small", bufs=3) as small,
    ):
        # ---- constants / weights ----
        wq_sb = consts.tile([128, KD, NHP, E], BF, tag="wq")
        wk_sb = consts.tile([128, KD, NHP, E], BF, tag="wk")
        wv_sb = consts.tile([128, KD, NHP, E], BF, tag="wv")
        nc.sync.dma_start(out=wq_sb,
                          in_=wqT.rearrange("(k p) hp e -> p k hp e", p=128))
        nc.sync.dma_start(out=wk_sb,
                          in_=wkT.rearrange("(k p) hp e -> p k hp e", p=128))
        nc.sync.dma_start(out=wv_sb,
                          in_=wvT.rearrange("(k p) hp e -> p k hp e", p=128))
        wo_sb = consts.tile([E, NHP, D], BF, tag="wo")
        nc.sync.dma_start(out=wo_sb, in_=woT)
        bcol_sb = consts.tile([E, NHP, 3], F32, tag="bcol")
        nc.sync.dma_start(out=bcol_sb, in_=bcol)
        cos_sb = consts.tile([E, N], BF, tag="cos")
        sin_sb = consts.tile([E, N], BF, tag="sin")
        nc.sync.dma_start(out=cos_sb, in_=cosb)
        nc.sync.dma_start(out=sin_sb, in_=sinb)
        rot_sb = consts.tile([E, E], BF, tag="rot")
        nc.sync.dma_start(out=rot_sb, in_=rotT)
        id_sb = consts.tile([E, E], BF, tag="idT")
        nc.sync.dma_start(out=id_sb, in_=idT)
        ones1_sb = consts.tile([1, TQC], F32R, tag="ones1")
        nc.sync.dma_start(out=ones1_sb, in_=ones1)

        # ---- x^T resident (buffer reused later for expT) ----
        x_sb = xbig.tile([128, KD, N], BF, tag="big")
        xTr = xT.rearrange("(k p) t -> p k t", p=128)
        for ci in range(NSL):
            nc.sync.dma_start(out=x_sb[:, :, ci * 512:(ci + 1) * 512],
                              in_=xTr[:, :, ci * 512:(ci + 1) * 512])

        # ---- persistent activations ----
        q_sb = acts.tile([E, NHP, N], BF, tag="q_sb")
        k_sb = acts.tile([E, NHP, N], BF, tag="k_sb")
        q2 = acts.tile([E, NHP, N], BF, tag="q2")
        k2 = acts.tile([E, NHP, N], BF, tag="k2")
        # v: [tk 128, chunk 16, head4 4, 64+ones]
        v_sb = acts.tile([128, NKC, NHP * HPC, HD + 1], BF, tag="v_sb")
        # normalized attention output, stacked: [e 128, hp 2, tq 2048]
        on_sb = acts.tile([E, NHP, N], BF, tag="on_sb")

        nc.vector.memset(v_sb[:, :, :, HD:HD + 1], 1.0)

        # ================= phase 1: projections + rope =================
        with (
            tc.tile_pool(name="ps_qk", bufs=2, space="PSUM") as ps_qk,
            tc.tile_pool(name="ps_v", bufs=2, space="PSUM") as ps_v,
            tc.tile_pool(name="ps_r", bufs=2, space="PSUM") as ps_r,
            tc.tile_pool(name="ps_t", bufs=2, space="PSUM") as ps_t,
        ):
            for ci in range(NSL):
                sl = slice(ci * 512, (ci + 1) * 512)
                for hp in range(NHP):
                    # q / k projection + rope for this (slice, head pair)
                    for dst, w, bc, dst2 in ((q_sb, wq_sb, 0, q2),
                                             (k_sb, wk_sb, 1, k2)):
                        ps = ps_qk.tile([128, 512], F32, tag="ps_qk")
                        for k in range(KD):
                            nc.tensor.matmul(
                                ps, w[:, k, hp, :], x_sb[:, k, sl],
                                start=(k == 0), stop=(k == KD - 1))
                        nc.vector.tensor_scalar_add(
                            dst[:, hp, sl], ps, bcol_sb[:, hp, bc:bc + 1])
                        # rope: dst2 = dst*cos + (P@dst)*sin
                        psr = ps_r.tile([128, 512], F32, tag="ps_r")
                        nc.tensor.matmul(psr, rot_sb, dst[:, hp, sl],
                                         start=True, stop=True)
                        t1 = small.tile([128, 512], BF, tag="rope_t1")
                        nc.vector.tensor_mul(t1, dst[:, hp, sl],
                                             cos_sb[:, sl])
                        t2 = small.tile([128, 512], BF, tag="rope_t2")
                        nc.vector.tensor_mul(t2, psr, sin_sb[:, sl])
                        nc.vector.tensor_add(dst2[:, hp, sl], t1, t2)
                    # vT projection, then transpose per 128-token chunk
                    psv = ps_v.tile([128, 512], F32, tag="ps_v")
                    for k in range(KD):
                        nc.tensor.matmul(
                            psv, wv_sb[:, k, hp, :], x_sb[:, k, sl],
                            start=(k == 0), stop=(k == KD - 1))
                    vts = small.tile([128, 512], BF, tag="vts")
                    nc.vector.tensor_scalar_add(vts, psv,
                                                bcol_sb[:, hp, 2:3])
                    for s in range(4):
                        cv = ci * 4 + s
                        pst = ps_t.tile([128, 128], BF, tag="ps_t")
                        nc.tensor.transpose(
                            pst, vts[:, s * 128:(s + 1) * 128], id_sb)
                        nc.vector.tensor_copy(
                            v_sb[:, cv, hp * HPC:(hp + 1) * HPC, 0:HD], pst)

        # ========= phase 2+3: attention + output projection =========
        with (
            tc.tile_pool(name="ps_sc", bufs=2, space="PSUM") as ps_sc,
            tc.tile_pool(name="ps_o", bufs=2, space="PSUM") as ps_o,
        ):
            pending = []

            def norm_item(tqc, hp, h, ou2, rs):
                def emit():
                    rc = small.tile([1, TQC], F32, tag="recip", bufs=2,
                                    name=f"rc_{tqc}_{hp}_{h}")
                    nc.vector.reciprocal_approx_fast(out=rc, in_=rs)
                    rcr = small.tile([1, TQC], F32R, tag="recipr", bufs=2,
                                     name=f"rcr_{tqc}_{hp}_{h}")
                    with nc.allow_low_precision(reason="f32->f32r round"):
                        nc.vector.tensor_copy(rcr, rc)
                    rb = ps_sc.tile([128, TQC], F32, tag="ps_sc",
                                    name=f"rb_{tqc}_{hp}_{h}")
                    for nn in range(TQC // 512):
                        nc.tensor.matmul(
                            rb[:, nn * 512:(nn + 1) * 512], ones1_sb[:, 0:128],
                            rcr[:, nn * 512:(nn + 1) * 512],
                            start=True, stop=True)
                    rbs = small.tile([128, TQC], BF, tag="recipb", bufs=2,
                                     name=f"rbs_{tqc}_{hp}_{h}")
                    nc.vector.tensor_copy(rbs, rb)
                    hsl = slice(h * HD, (h + 1) * HD)
                    nc.vector.tensor_mul(
                        on_sb[hsl, hp, tqc * TQC:(tqc + 1) * TQC],
                        ou2[hsl, :], rbs[hsl, :])
                    if hp == NHP - 1 and h == HPC - 1:
                        for ci in range(tqc * 8, tqc * 8 + 8):
                            pending.append(y_item(ci))
                return emit

            def y_item(ci):
                def emit():
                    psy = ps_sc.tile([128, D], F32, tag="ps_sc",
                                     name=f"psy_{ci}")
                    for eo in range(D // 512):
                        for hp in range(NHP):
                            nc.tensor.matmul(
                                psy[:, eo * 512:(eo + 1) * 512],
                                on_sb[:, hp, ci * 128:(ci + 1) * 128],
                                wo_sb[:, hp, eo * 512:(eo + 1) * 512],
                                start=(hp == 0), stop=(hp == NHP - 1))
                    ysb = small.tile([128, D], BF, tag="ysb", bufs=2,
                                     name=f"ysb_{ci}")
                    nc.vector.tensor_copy(ysb, psy)
                    nc.sync.dma_start(
                        out=y[ci * 128:(ci + 1) * 128, :], in_=ysb)
                return emit

            for tqc in range(NTQC):
                for hp in range(NHP):
                    tq0 = tqc * TQC
                    exp_t = xbig.tile([128, HPC, NKC, TQC], BF, tag="big")
                    ots = [ps_o.tile([HD + 1, TQC], F32, tag="ps_o",
                                     name=f"ot_{tqc}_{hp}_{h}")
                           for h in range(HPC)]

                    def attn_mms(j):
                        for h in range(HPC):
                            vt = v_sb[:, j, hp * HPC + h, :]
                            for nn in range(TQC // 512):
                                nc.tensor.matmul(
                                    ots[h][:, nn * 512:(nn + 1) * 512], vt,
                                    exp_t[:, h, j, nn * 512:(nn + 1) * 512],
                                    start=(j == 0), stop=(j == NKC - 1))

                    for tkc in range(NKC):
                        scs = []
                        for h in range(HPC):
                            sc = ps_sc.tile([128, TQC], F32, tag="ps_sc")
                            lhsT = k2[h * HD:(h + 1) * HD, hp,
                                      tkc * 128:(tkc + 1) * 128]
                            for nn in range(TQC // 512):
                                nc.tensor.matmul(
                                    sc[:, nn * 512:(nn + 1) * 512], lhsT,
                                    q2[h * HD:(h + 1) * HD, hp,
                                       tq0 + nn * 512: tq0 + (nn + 1) * 512],
                                    start=True, stop=True)
                            scs.append(sc)
                        for h in range(HPC):
                            nc.scalar.activation(
                                out=exp_t[:, h, tkc, :], in_=scs[h],
                                func=mybir.ActivationFunctionType.Exp,
                                scale=float(HD) ** -0.5)
                        if tkc > 0:
                            attn_mms(tkc - 1)
                        if tkc >= 1 and pending:
                            pending.pop(0)()
                    attn_mms(NKC - 1)

                    # Evacuate ot PSUM tiles; rows 0..63 = out, row 64 = denom
                    ou2 = small.tile([128, TQC], BF, tag="ou", bufs=2,
                                     name=f"ou_{tqc}_{hp}")
                    nc.vector.tensor_copy(ou2[0:HD, :], ots[0][0:HD, :])
                    nc.vector.tensor_copy(ou2[HD:2 * HD, :], ots[1][0:HD, :])
                    rss = []
                    for h in range(HPC):
                        rs = small.tile([1, TQC], F32, tag="rs", bufs=4,
                                        name=f"rs_{tqc}_{hp}_{h}")
                        nc.vector.tensor_copy(rs, ots[h][HD:HD + 1, :])
                        rss.append(rs)
                    for h in range(HPC):
                        pending.append(norm_item(tqc, hp, h, ou2, rss[h]))

            while pending:
                pending.pop(0)()


def _host_inputs(x, Wq, Wk, Wv, Wo, bq, bk, bv, bo):
    """Build the 8 per-core input maps (host-side sharding + layout prep)."""
    bf16 = ml_dtypes.bfloat16

    # rope tables: row e uses freq (e % 64) % 32
    i = (np.arange(E) % HD) % (HD // 2)
    inv_freq = ROPE_BASE ** (-2.0 * i / HD)  # [E]
    ang = np.arange(N)[None, :] * inv_freq[:, None]          # [E, N]
    cosb = np.cos(ang).astype(bf16)
    sinb = np.sin(ang).astype(bf16)
    # rotate-half permutation: rot = P @ q (per 64-block)
    P = np.zeros((E, E), dtype=np.float32)
    for h in range(HPC):
        for j in range(HD // 2):
            P[h * HD + j, h * HD + j + HD // 2] = -1.0
            P[h * HD + j + HD // 2, h * HD + j] = 1.0
    rotT = np.ascontiguousarray(P.T).astype(bf16)
    idT = np.eye(E, dtype=np.float32).astype(bf16)
    ones1 = np.ones((1, TQC), dtype=np.float32)

    xb = [np.ascontiguousarray(x[b].T).astype(bf16) for b in range(B)]

    in_maps = []
    for c in range(NCORES):
        b, g = c // 4, c % 4
        sl = slice(g * 256, (g + 1) * 256)

        def wsplit(W):
            # [D, 2, 128]: head-pair split of this quad's columns of W.T
            wt = W[sl, :].T.reshape(D, NHP, E)
            return np.ascontiguousarray(wt).astype(bf16)

        in_maps.append({
            "xT": xb[b],
            "wqT": wsplit(Wq),
            "wkT": wsplit(Wk),
            "wvT": wsplit(Wv),
            "woT": np.ascontiguousarray(
                Wo[:, sl].T.reshape(NHP, E, D).transpose(1, 0, 2))
                .astype(bf16),
            "bcol": np.stack([bq[sl], bk[sl], bv[sl]], axis=1)
                .reshape(NHP, E, 3).transpose(1, 0, 2)
                .astype(np.float32).copy(),
            "cosb": cosb,
            "sinb": sinb,
            "rotT": rotT,
            "idT": idT,
            "ones1": ones1,
        })
    return in_maps


_NC = None


def gather(results, bo):
    """Sum the 8 per-core partial y's into the full [B, N, D] output."""
    out = np.zeros((B, N, D), dtype=np.float32)
    for c, r in enumerate(results):
        out[c // 4] += np.asarray(r["y"], dtype=np.float32)
    out += np.asarray(bo, dtype=np.float32)[None, None, :]
    return out


def kernel(x, Wq, Wk, Wv, Wo, bq, bk, bv, bo):
    from concourse.bass_utils import run_bass_kernel_spmd

    global _NC
    if _NC is None:
        _NC = build_nc()
    in_maps = _host_inputs(np.asarray(x, dtype=np.float32),
                           np.asarray(Wq, dtype=np.float32),
                           np.asarray(Wk, dtype=np.float32),
                           np.asarray(Wv, dtype=np.float32),
                           np.asarray(Wo, dtype=np.float32),
                           np.asarray(bq, dtype=np.float32),
                           np.asarray(bk, dtype=np.float32),
                           np.asarray(bv, dtype=np.float32),
                           np.asarray(bo, dtype=np.float32))
    res = run_bass_kernel_spmd(_NC, in_maps, core_ids=list(range(NCORES)))
    return gather(res.results, np.asarray(bo, dtype=np.float32))


# revision 22
# speedup vs baseline: 1.0842x; 1.0842x over previous
# Multi-head attention with RoPE, tensor-parallel over heads on 8 NeuronCores.
#
# Problem: B=2, N=2048, D=1024, H=16 heads, head_dim=64.
#   q/k/v = x @ W{q,k,v}.T + b;  RoPE(q), RoPE(k);  softmax(q k^T / 8) v;
#   out = attn @ Wo.T + bo.
#
# Sharding: 2 heads per core (column-parallel QKV, row-parallel out-proj).
# Each core emits a partial [T, D] output (bf16); host sums the 8 partials
# and adds bo.
#
# Per-core layout strategy (all matmul inputs bf16, fp32 accumulation):
#   xT   [D, T]    : x transposed on host, so the contraction dim d is on
#                    partitions for every projection matmul.
#   q,k  [E=128, T]: "transposed" activations (2 heads * 64 on partitions).
#                    Biases folded in via tensor_scalar_add during the
#                    PSUM->SBUF evacuation (per-partition scalar).
#   RoPE           : rotate-half done as a 128x128 constant permutation matmul
#                    (rot = P @ q), then q'' = q*cos + rot*sin on DVE.
#   v              : projected as vT [E, T] (tokens moving, N=512 per matmul),
#                    bias folded on evacuation, then PE-transposed per
#                    128-token chunk into v_sb [tk, 130] = [1 | v_h0 | v_h1 | 1]
#                    so attn@v also yields softmax row sums (ones columns).
#   scores^T       : [tk, tq] via lhsT=k (so exp output feeds attn@v directly
#                    as the moving operand, free dim 512).
#   attn@v         : oT[65, tq] = [1|v_h]^T @ expT, accumulated over tk in
#                    PSUM; h0: row 0 = denom, rows 1-64 = out; h1: rows 0-63 =
#                    out, row 64 = denom.
#   normalize      : reciprocal_approx_fast of the denom row, broadcast to 128
#                    partitions via a K=1 matmul, multiplied into the stacked
#                    on_sb [128, b, tq] during the PSUM->SBUF copy.
#   out-proj       : y[t, :] = on^T @ woT per 128-token tile (single K=128
#                    matmul per 512-col chunk; both heads contracted at once).
#                    bo is added by the host after the partial-sum gather.

import numpy as np
import ml_dtypes

import concourse.bass as bass
import concourse.mybir as mybir
import concourse.tile as tile
from concourse import bacc

B, N, D, H = 2, 2048, 1024, 16
HD = 64
T = B * N                 # 4096 tokens
NCORES = 8
HPC = H // NCORES         # 2 heads per core
E = HPC * HD              # 128 per-core projection columns
KD = D // 128             # 8 contraction tiles for d
ROPE_BASE = 10000.0

BF = mybir.dt.bfloat16
F32 = mybir.dt.float32
F32R = mybir.dt.float32r

TQC = 1024                # tq chunk (exp granularity / psum width)
NTQC = N // TQC           # 2 per batch
NKC = N // 128            # 16 key chunks per batch


def build_nc():
    nc = bacc.Bacc(trn_type="TRN2", target_bir_lowering=False, debug=False)

    xT = nc.dram_tensor("xT", [D, T], BF, kind="ExternalInput").ap()
    wqT = nc.dram_tensor("wqT", [D, E], BF, kind="ExternalInput").ap()
    wkT = nc.dram_tensor("wkT", [D, E], BF, kind="ExternalInput").ap()
    wvT = nc.dram_tensor("wvT", [D, E], BF, kind="ExternalInput").ap()
    woT = nc.dram_tensor("woT", [E, D], BF, kind="ExternalInput").ap()
    bcol = nc.dram_tensor("bcol", [E, 3], F32, kind="ExternalInput").ap()
    cosb = nc.dram_tensor("cosb", [E, N], BF, kind="ExternalInput").ap()
    sinb = nc.dram_tensor("sinb", [E, N], BF, kind="ExternalInput").ap()
    rotT = nc.dram_tensor("rotT", [E, E], BF, kind="ExternalInput").ap()
    idT = nc.dram_tensor("idT", [E, E], BF, kind="ExternalInput").ap()
    ones1 = nc.dram_tensor("ones1", [1, TQC], F32R, kind="ExternalInput").ap()
    y = nc.dram_tensor("y", [T, D], BF, kind="ExternalOutput").ap()

    with tile.TileContext(nc) as tc:
        _build(tc, nc, xT, wqT, wkT, wvT, woT, bcol, cosb, sinb,
               rotT, idT, ones1, y)
    nc.compile()  # bacc legalization: splits multi-wait instructions etc.
    return nc


def _build(tc, nc, xT, wqT, wkT, wvT, woT, bcol, cosb, sinb,
           rotT, idT, ones1, y):
    with (
        tc.tile_pool(name="consts", bufs=1) as consts,
        tc.tile_pool(name="xbig", bufs=1) as xbig,
        tc.tile_pool(name="acts", bufs=1) as acts,
        tc.tile_pool(name="small", bufs=3) as small,
    ):
        # ---- constants / weights ----
        wq_sb = consts.tile([128, KD, E], BF, tag="wq")
        wk_sb = consts.tile([128, KD, E], BF, tag="wk")
        wv_sb = consts.tile([128, KD, E], BF, tag="wv")
        nc.sync.dma_start(out=wq_sb, in_=wqT.rearrange("(k p) e -> p k e", p=128))
        nc.sync.dma_start(out=wk_sb, in_=wkT.rearrange("(k p) e -> p k e", p=128))
        nc.sync.dma_start(out=wv_sb, in_=wvT.rearrange("(k p) e -> p k e", p=128))
        wo_sb = consts.tile([E, D], BF, tag="wo")
        nc.sync.dma_start(out=wo_sb, in_=woT)
        bcol_sb = consts.tile([E, 3], F32, tag="bcol")
        nc.sync.dma_start(out=bcol_sb, in_=bcol)
        cos_sb = consts.tile([E, N], BF, tag="cos")
        sin_sb = consts.tile([E, N], BF, tag="sin")
        nc.sync.dma_start(out=cos_sb, in_=cosb)
        nc.sync.dma_start(out=sin_sb, in_=sinb)
        rot_sb = consts.tile([E, E], BF, tag="rot")
        nc.sync.dma_start(out=rot_sb, in_=rotT)
        id_sb = consts.tile([E, E], BF, tag="idT")
        nc.sync.dma_start(out=id_sb, in_=idT)
        ones1_sb = consts.tile([1, TQC], F32R, tag="ones1")
        nc.sync.dma_start(out=ones1_sb, in_=ones1)

        # ---- x^T resident (reused later for expT) ----
        # 8 t-sliced DMAs so the first q/k matmuls start after ~1/8 of x.
        x_sb = xbig.tile([128, KD, T], BF, tag="big")
        xTr = xT.rearrange("(k p) t -> p k t", p=128)
        for ci in range(T // 512):
            nc.sync.dma_start(out=x_sb[:, :, ci * 512:(ci + 1) * 512],
                              in_=xTr[:, :, ci * 512:(ci + 1) * 512])

        # ---- persistent activations ----
        q_sb = acts.tile([E, T], BF, tag="q_sb")
        k_sb = acts.tile([E, T], BF, tag="k_sb")
        q2 = acts.tile([E, T], BF, tag="q2")
        k2 = acts.tile([E, T], BF, tag="k2")
        # v tiles: [tk 128, tk-chunk 32, head 2, 64+ones] = [v_h | 1] per head
        v_sb = acts.tile([128, T // 128, HPC, HD + 1], BF, tag="v_sb")
        # normalized attention output, stacked: [e 128, b 2, tq 2048]
        on_sb = acts.tile([E, B, N], BF, tag="on_sb")

        nc.vector.memset(v_sb[:, :, :, HD:HD + 1], 1.0)

        # ================= phase 1: projections + rope =================
        with (
            tc.tile_pool(name="ps_qk", bufs=2, space="PSUM") as ps_qk,
            tc.tile_pool(name="ps_v", bufs=2, space="PSUM") as ps_v,
            tc.tile_pool(name="ps_r", bufs=2, space="PSUM") as ps_r,
            tc.tile_pool(name="ps_t", bufs=2, space="PSUM") as ps_t,
        ):
            # Emission interleaved per 512-token slice so compute tracks the
            # incoming x DMA stream and the PE never sits idle long.
            for ci in range(T // 512):
                sl = slice(ci * 512, (ci + 1) * 512)
                npos = (ci * 512) % N
                tsl = slice(npos, npos + 512)
                # q / k projection + rope for this slice
                for dst, w, bc, dst2 in ((q_sb, wq_sb, 0, q2),
                                         (k_sb, wk_sb, 1, k2)):
                    ps = ps_qk.tile([128, 512], F32, tag="ps_qk")
                    for k in range(KD):
                        nc.tensor.matmul(
                            ps, w[:, k, :], x_sb[:, k, sl],
                            start=(k == 0), stop=(k == KD - 1))
                    nc.vector.tensor_scalar_add(
                        dst[:, sl], ps, bcol_sb[:, bc:bc + 1])
                    # rope: dst2 = dst*cos + (P@dst)*sin
                    psr = ps_r.tile([128, 512], F32, tag="ps_r")
                    nc.tensor.matmul(psr, rot_sb, dst[:, sl],
                                     start=True, stop=True)
                    t1 = small.tile([128, 512], BF, tag="rope_t1")
                    nc.vector.tensor_mul(t1, dst[:, sl], cos_sb[:, tsl])
                    t2 = small.tile([128, 512], BF, tag="rope_t2")
                    nc.vector.tensor_mul(t2, psr, sin_sb[:, tsl])
                    nc.vector.tensor_add(dst2[:, sl], t1, t2)
                # vT projection for this slice, then transpose per 128 tokens
                psv = ps_v.tile([128, 512], F32, tag="ps_v")
                for k in range(KD):
                    nc.tensor.matmul(
                        psv, wv_sb[:, k, :], x_sb[:, k, sl],
                        start=(k == 0), stop=(k == KD - 1))
                vts = small.tile([128, 512], BF, tag="vts")
                nc.vector.tensor_scalar_add(vts, psv, bcol_sb[:, 2:3])
                for s in range(4):
                    cv = ci * 4 + s
                    pst = ps_t.tile([128, 128], BF, tag="ps_t")
                    nc.tensor.transpose(
                        pst, vts[:, s * 128:(s + 1) * 128], id_sb)
                    # strided write: head h's 64 v-columns at [cv, h, 0:64]
                    nc.vector.tensor_copy(v_sb[:, cv, :, 0:HD], pst)

        # ========= phase 2+3: attention + output projection =========
        # Per (b, tqc): for each key chunk tkc, scores (both heads,
        # row-packed) -> exp -> attn@v MMs, interleaved so the PE always has
        # attn work for chunk tkc while ACT computes exp for chunk tkc+1.
        with (
            tc.tile_pool(name="ps_sc", bufs=2, space="PSUM") as ps_sc,
            tc.tile_pool(name="ps_o", bufs=2, space="PSUM") as ps_o,
        ):
            # Deferred-work queue: each block's normalization chain and the
            # per-batch output-projection chunks are emitted one-per-tkc
            # inside LATER blocks' loops, so the scores/exp/attn drum never
            # pauses.
            pending = []

            def norm_item(b, tqc, h, ou2, rs):
                # ou2: stacked unnormalized attn out [128, TQC] bf16 (SBUF);
                #      this head's rows live at partitions h*64..h*64+63
                # rs: softmax denominators [1, TQC] f32 (SBUF)
                def emit():
                    rc = small.tile([1, TQC], F32, tag="recip", bufs=2,
                                    name=f"rc_{b}_{tqc}_{h}")
                    nc.vector.reciprocal_approx_fast(out=rc, in_=rs)
                    rcr = small.tile([1, TQC], F32R, tag="recipr", bufs=2,
                                     name=f"rcr_{b}_{tqc}_{h}")
                    with nc.allow_low_precision(reason="f32->f32r round"):
                        nc.vector.tensor_copy(rcr, rc)
                    rb = ps_sc.tile([128, TQC], F32, tag="ps_sc",
                                    name=f"rb_{b}_{tqc}_{h}")
                    for nn in range(TQC // 512):
                        nc.tensor.matmul(
                            rb[:, nn * 512:(nn + 1) * 512], ones1_sb[:, 0:128],
                            rcr[:, nn * 512:(nn + 1) * 512],
                            start=True, stop=True)
                    rbs = small.tile([128, TQC], BF, tag="recipb", bufs=2,
                                     name=f"rbs_{b}_{tqc}_{h}")
                    nc.vector.tensor_copy(rbs, rb)
                    hsl = slice(h * HD, (h + 1) * HD)
                    nc.vector.tensor_mul(
                        on_sb[hsl, b, tqc * TQC:(tqc + 1) * TQC],
                        ou2[hsl, :], rbs[hsl, :])
                    if h == HPC - 1:
                        # both heads of (b, tqc) normalized -> the matching
                        # output-projection chunks are now eligible
                        for ci in range(tqc * 8, tqc * 8 + 8):
                            pending.append(y_item(b, ci))
                return emit

            def y_item(b, ci):
                def emit():
                    psy = ps_sc.tile([128, D], F32, tag="ps_sc",
                                     name=f"psy_{b}_{ci}")
                    for eo in range(D // 512):
                        nc.tensor.matmul(
                            psy[:, eo * 512:(eo + 1) * 512],
                            on_sb[:, b, ci * 128:(ci + 1) * 128],
                            wo_sb[:, eo * 512:(eo + 1) * 512],
                            start=True, stop=True)
                    ysb = small.tile([128, D], BF, tag="ysb", bufs=2,
                                     name=f"ysb_{b}_{ci}")
                    nc.vector.tensor_copy(ysb, psy)
                    nc.sync.dma_start(
                        out=y[b * N + ci * 128: b * N + (ci + 1) * 128, :],
                        in_=ysb)
                return emit

            for b in range(B):
                for tqc in range(NTQC):
                    tq0 = b * N + tqc * TQC  # global tq base
                    exp_t = xbig.tile([128, HPC, NKC, TQC], BF, tag="big")
                    ots = [ps_o.tile([HD + 1, TQC], F32, tag="ps_o",
                                     name=f"ot_{b}_{tqc}_{h}")
                           for h in range(HPC)]

                    def attn_mms(j):
                        # attn@v for key chunk j (consumes exp_t[:, :, j, :])
                        for h in range(HPC):
                            vt = v_sb[:, b * NKC + j, h, :]
                            for nn in range(TQC // 512):
                                nc.tensor.matmul(
                                    ots[h][:, nn * 512:(nn + 1) * 512], vt,
                                    exp_t[:, h, j, nn * 512:(nn + 1) * 512],
                                    start=(j == 0), stop=(j == NKC - 1))

                    # Software-pipelined: the PE's attn@v for chunk tkc-1 is
                    # emitted after exp(tkc) so the PE never waits on the
                    # same-iteration exp; ACT (exp) is the steady-state drum.
                    # One deferred item (prev block's normalization / y-proj
                    # chunk) is woven in per tkc iteration.
                    for tkc in range(NKC):
                        scs = []
                        for h in range(HPC):
                            sc = ps_sc.tile([128, TQC], F32, tag="ps_sc")
                            lhsT = k2[h * HD:(h + 1) * HD,
                                      b * N + tkc * 128: b * N + (tkc + 1) * 128]
                            for nn in range(TQC // 512):
                                nc.tensor.matmul(
                                    sc[:, nn * 512:(nn + 1) * 512], lhsT,
                                    q2[h * HD:(h + 1) * HD,
                                       tq0 + nn * 512: tq0 + (nn + 1) * 512],
                                    start=True, stop=True)
                            scs.append(sc)
                        for h in range(HPC):
                            nc.scalar.activation(
                                out=exp_t[:, h, tkc, :], in_=scs[h],
                                func=mybir.ActivationFunctionType.Exp,
                                scale=float(HD) ** -0.5)
                        if tkc > 0:
                            attn_mms(tkc - 1)
                        if tkc >= 1 and pending:
                            pending.pop(0)()
                    attn_mms(NKC - 1)

                    # Evacuate the ot PSUM tiles quickly so the next block's
                    # attn matmuls get the slots; the reciprocal chain is
                    # deferred via `pending`.
                    # Each head: psum rows 0..63 = out, row 64 = denom.
                    ou2 = small.tile([128, TQC], BF, tag="ou", bufs=2,
                                     name=f"ou_{b}_{tqc}")
                    nc.vector.tensor_copy(ou2[0:HD, :], ots[0][0:HD, :])
                    nc.vector.tensor_copy(ou2[HD:2 * HD, :], ots[1][0:HD, :])
                    rss = []
                    for h in range(HPC):
                        rs = small.tile([1, TQC], F32, tag="rs", bufs=4,
                                        name=f"rs_{b}_{tqc}_{h}")
                        nc.vector.tensor_copy(rs, ots[h][HD:HD + 1, :])
                        rss.append(rs)
                    for h in range(HPC):
                        pending.append(norm_item(b, tqc, h, ou2, rss[h]))

            # drain remaining deferred work (last block's norms + final ys)
            while pending:
                pending.pop(0)()


def _host_inputs(x, Wq, Wk, Wv, Wo, bq, bk, bv, bo):
    """Build the 8 per-core input maps (host-side sharding + layout prep)."""
    bf16 = ml_dtypes.bfloat16
    xTh = np.ascontiguousarray(x.reshape(T, D).T).astype(bf16)

    # rope tables: row e uses freq (e % 64) % 32; positions along columns
    i = (np.arange(E) % HD) % (HD // 2)
    inv_freq = ROPE_BASE ** (-2.0 * i / HD)  # [E]
    ang = np.arange(N)[None, :] * inv_freq[:, None]          # [E, N]
    cosb = np.cos(ang).astype(bf16)
    sinb = np.sin(ang).astype(bf16)

    # rotate-half permutation: rot = P @ q (per 64-block)
    P = np.zeros((E, E), dtype=np.float32)
    for h in range(HPC):
        for j in range(HD // 2):
            P[h * HD + j, h * HD + j + HD // 2] = -1.0
            P[h * HD + j + HD // 2, h * HD + j] = 1.0
    rotT = np.ascontiguousarray(P.T).astype(bf16)
    idT = np.eye(E, dtype=np.float32).astype(bf16)

    ones1 = np.ones((1, TQC), dtype=np.float32)

    in_maps = []
    for c in range(NCORES):
        sl = slice(c * E, (c + 1) * E)
        in_maps.append({
            "xT": xTh,
            "wqT": np.ascontiguousarray(Wq[sl, :].T).astype(bf16),
            "wkT": np.ascontiguousarray(Wk[sl, :].T).astype(bf16),
            "wvT": np.ascontiguousarray(Wv[sl, :].T).astype(bf16),
            "woT": np.ascontiguousarray(Wo[:, sl].T).astype(bf16),
            "bcol": np.stack([bq[sl], bk[sl], bv[sl]], axis=1)
                .astype(np.float32),
            "cosb": cosb,
            "sinb": sinb,
            "rotT": rotT,
            "idT": idT,
            "ones1": ones1,
        })
    return in_maps


_NC = None


def kernel(x, Wq, Wk, Wv, Wo, bq, bk, bv, bo):
    from concourse.bass_utils import run_bass_kernel_spmd

    global _NC
    if _NC is None:
        _NC = build_nc()
    bo = np.asarray(bo, dtype=np.float32)
    in_maps = _host_inputs(np.asarray(x, dtype=np.float32),
                           np.asarray(Wq, dtype=np.float32),
                           np.asarray(Wk, dtype=np.float32),
                           np.asarray(Wv, dtype=np.float32),
                           np.asarray(Wo, dtype=np.float32),
                           np.asarray(bq, dtype=np.float32),
                           np.asarray(bk, dtype=np.float32),
                           np.asarray(bv, dtype=np.float32),
                           bo)
    res = run_bass_kernel_spmd(_NC, in_maps, core_ids=list(range(NCORES)))
    out = np.zeros((T, D), dtype=np.float32)
    for r in res.results:
        out += np.asarray(r["y"], dtype=np.float32)
    out += bo[None, :]
    return out.reshape(B, N, D)


# revision 23
# speedup vs baseline: 1.1548x; 1.0651x over previous
# Multi-head attention with RoPE, tensor-parallel over heads on 8 NeuronCores.
#
# Problem: B=2, N=2048, D=1024, H=16 heads, head_dim=64.
#   q/k/v = x @ W{q,k,v}.T + b;  RoPE(q), RoPE(k);  softmax(q k^T / 8) v;
#   out = attn @ Wo.T + bo.
#
# Sharding: 2 heads per core (column-parallel QKV, row-parallel out-proj).
# Each core emits a partial [T, D] output (bf16); host sums the 8 partials
# and adds bo.
#
# Per-core layout strategy (all matmul inputs bf16, fp32 accumulation):
#   xT   [D, T]    : x transposed on host, so the contraction dim d is on
#                    partitions for every projection matmul.
#   q,k  [E=128, T]: "transposed" activations (2 heads * 64 on partitions).
#                    Biases folded in via tensor_scalar_add during the
#                    PSUM->SBUF evacuation (per-partition scalar).
#   RoPE           : rotate-half done as a 128x128 constant permutation matmul
#                    (rot = P @ q), then q'' = q*cos + rot*sin on DVE.
#   v              : projected as vT [E, T] (tokens moving, N=512 per matmul),
#                    bias folded on evacuation, then PE-transposed per
#                    128-token chunk into v_sb [tk, 130] = [1 | v_h0 | v_h1 | 1]
#                    so attn@v also yields softmax row sums (ones columns).
#   scores^T       : [tk, tq] via lhsT=k (so exp output feeds attn@v directly
#                    as the moving operand, free dim 512).
#   attn@v         : oT[65, tq] = [1|v_h]^T @ expT, accumulated over tk in
#                    PSUM; h0: row 0 = denom, rows 1-64 = out; h1: rows 0-63 =
#                    out, row 64 = denom.
#   normalize      : reciprocal_approx_fast of the denom row, broadcast to 128
#                    partitions via a K=1 matmul, multiplied into the stacked
#                    on_sb [128, b, tq] during the PSUM->SBUF copy.
#   out-proj       : y[t, :] = on^T @ woT per 128-token tile (single K=128
#                    matmul per 512-col chunk; both heads contracted at once).
#                    bo is added by the host after the partial-sum gather.

import numpy as np
import ml_dtypes

import concourse.bass as bass
import concourse.mybir as mybir
import concourse.tile as tile
from concourse import bacc

B, N, D, H = 2, 2048, 1024, 16
HD = 64
T = B * N                 # 4096 tokens
NCORES = 8
HPC = H // NCORES         # 2 heads per core
E = HPC * HD              # 128 per-core projection columns
KD = D // 128             # 8 contraction tiles for d
ROPE_BASE = 10000.0

BF = mybir.dt.bfloat16
F32 = mybir.dt.float32
F32R = mybir.dt.float32r

TQC = 1024                # tq chunk (exp granularity / psum width)
NTQC = N // TQC           # 2 per batch
NKC = N // 128            # 16 key chunks per batch


def build_nc():
    nc = bacc.Bacc(trn_type="TRN2", target_bir_lowering=False, debug=False)

    xT = nc.dram_tensor("xT", [D, T], BF, kind="ExternalInput").ap()
    wqT = nc.dram_tensor("wqT", [D, E], BF, kind="ExternalInput").ap()
    wkT = nc.dram_tensor("wkT", [D, E], BF, kind="ExternalInput").ap()
    wvT = nc.dram_tensor("wvT", [D, E], BF, kind="ExternalInput").ap()
    woT = nc.dram_tensor("woT", [E, D], BF, kind="ExternalInput").ap()
    bcol = nc.dram_tensor("bcol", [E, 3], F32, kind="ExternalInput").ap()
    cosb = nc.dram_tensor("cosb", [E, N], BF, kind="ExternalInput").ap()
    sinb = nc.dram_tensor("sinb", [E, N], BF, kind="ExternalInput").ap()
    rotT = nc.dram_tensor("rotT", [E, E], BF, kind="ExternalInput").ap()
    idT = nc.dram_tensor("idT", [E, E], BF, kind="ExternalInput").ap()
    ones1 = nc.dram_tensor("ones1", [1, TQC], F32R, kind="ExternalInput").ap()
    y = nc.dram_tensor("y", [T, D], BF, kind="ExternalOutput").ap()

    with tile.TileContext(nc) as tc:
        _build(tc, nc, xT, wqT, wkT, wvT, woT, bcol, cosb, sinb,
               rotT, idT, ones1, y)
    nc.compile()  # bacc legalization: splits multi-wait instructions etc.
    return nc


def _build(tc, nc, xT, wqT, wkT, wvT, woT, bcol, cosb, sinb,
           rotT, idT, ones1, y):
    with (
        tc.tile_pool(name="consts", bufs=1) as consts,
        tc.tile_pool(name="xbig", bufs=1) as xbig,
        tc.tile_pool(name="acts", bufs=1) as acts,
        tc.tile_pool(name="small", bufs=3) as small,
    ):
        # ---- constants / weights ----
        wq_sb = consts.tile([128, KD, E], BF, tag="wq")
        wk_sb = consts.tile([128, KD, E], BF, tag="wk")
        wv_sb = consts.tile([128, KD, E], BF, tag="wv")
        nc.sync.dma_start(out=wq_sb, in_=wqT.rearrange("(k p) e -> p k e", p=128))
        nc.sync.dma_start(out=wk_sb, in_=wkT.rearrange("(k p) e -> p k e", p=128))
        nc.sync.dma_start(out=wv_sb, in_=wvT.rearrange("(k p) e -> p k e", p=128))
        wo_sb = consts.tile([E, D], BF, tag="wo")
        nc.sync.dma_start(out=wo_sb, in_=woT)
        bcol_sb = consts.tile([E, 3], F32, tag="bcol")
        nc.sync.dma_start(out=bcol_sb, in_=bcol)
        cos_sb = consts.tile([E, N], BF, tag="cos")
        sin_sb = consts.tile([E, N], BF, tag="sin")
        nc.sync.dma_start(out=cos_sb, in_=cosb)
        nc.sync.dma_start(out=sin_sb, in_=sinb)
        rot_sb = consts.tile([E, E], BF, tag="rot")
        nc.sync.dma_start(out=rot_sb, in_=rotT)
        id_sb = consts.tile([E, E], BF, tag="idT")
        nc.sync.dma_start(out=id_sb, in_=idT)
        ones1_sb = consts.tile([1, TQC], F32R, tag="ones1")
        nc.sync.dma_start(out=ones1_sb, in_=ones1)

        # ---- x^T resident (reused later for expT) ----
        # 8 t-sliced DMAs so the first q/k matmuls start after ~1/8 of x.
        x_sb = xbig.tile([128, KD, T], BF, tag="big")
        xTr = xT.rearrange("(k p) t -> p k t", p=128)
        for ci in range(T // 512):
            nc.sync.dma_start(out=x_sb[:, :, ci * 512:(ci + 1) * 512],
                              in_=xTr[:, :, ci * 512:(ci + 1) * 512])

        # ---- persistent activations ----
        q_sb = acts.tile([E, T], BF, tag="q_sb")
        k_sb = acts.tile([E, T], BF, tag="k_sb")
        q2 = acts.tile([E, T], BF, tag="q2")
        k2 = acts.tile([E, T], BF, tag="k2")
        # v tiles: [tk 128, tk-chunk 32, head 2, 64+ones] = [v_h | 1] per head
        v_sb = acts.tile([128, T // 128, HPC, HD + 1], BF, tag="v_sb")
        # normalized attention output, stacked: [e 128, b 2, tq 2048]
        on_sb = acts.tile([E, B, N], BF, tag="on_sb")

        nc.vector.memset(v_sb[:, :, :, HD:HD + 1], 1.0)

        # ================= phase 1: projections + rope =================
        with (
            tc.tile_pool(name="ps_qk", bufs=2, space="PSUM") as ps_qk,
            tc.tile_pool(name="ps_v", bufs=2, space="PSUM") as ps_v,
            tc.tile_pool(name="ps_r", bufs=2, space="PSUM") as ps_r,
            tc.tile_pool(name="ps_t", bufs=2, space="PSUM") as ps_t,
        ):
            # Emission interleaved per 512-token slice so compute tracks the
            # incoming x DMA stream and the PE never sits idle long.
            for ci in range(T // 512):
                sl = slice(ci * 512, (ci + 1) * 512)
                npos = (ci * 512) % N
                tsl = slice(npos, npos + 512)
                # q / k projection + rope for this slice
                for dst, w, bc, dst2 in ((q_sb, wq_sb, 0, q2),
                                         (k_sb, wk_sb, 1, k2)):
                    ps = ps_qk.tile([128, 512], F32, tag="ps_qk")
                    for k in range(KD):
                        nc.tensor.matmul(
                            ps, w[:, k, :], x_sb[:, k, sl],
                            start=(k == 0), stop=(k == KD - 1))
                    nc.vector.tensor_scalar_add(
                        dst[:, sl], ps, bcol_sb[:, bc:bc + 1])
                    # rope: dst2 = dst*cos + (P@dst)*sin
                    psr = ps_r.tile([128, 512], F32, tag="ps_r")
                    nc.tensor.matmul(psr, rot_sb, dst[:, sl],
                                     start=True, stop=True)
                    t1 = small.tile([128, 512], BF, tag="rope_t1")
                    nc.vector.tensor_mul(t1, dst[:, sl], cos_sb[:, tsl])
                    t2 = small.tile([128, 512], BF, tag="rope_t2")
                    nc.vector.tensor_mul(t2, psr, sin_sb[:, tsl])
                    nc.vector.tensor_add(dst2[:, sl], t1, t2)
                # vT projection for this slice, then transpose per 128 tokens
                psv = ps_v.tile([128, 512], F32, tag="ps_v")
                for k in range(KD):
                    nc.tensor.matmul(
                        psv, wv_sb[:, k, :], x_sb[:, k, sl],
                        start=(k == 0), stop=(k == KD - 1))
                vts = small.tile([128, 512], BF, tag="vts")
                nc.vector.tensor_scalar_add(vts, psv, bcol_sb[:, 2:3])
                for s in range(4):
                    cv = ci * 4 + s
                    pst = ps_t.tile([128, 128], BF, tag="ps_t")
                    nc.tensor.transpose(
                        pst, vts[:, s * 128:(s + 1) * 128], id_sb)
                    # strided write: head h's 64 v-columns at [cv, h, 0:64]
                    nc.vector.tensor_copy(v_sb[:, cv, :, 0:HD], pst)

        # ========= phase 2+3: attention + output projection =========
        # Per (b, tqc): for each key chunk tkc, scores (both heads,
        # row-packed) -> exp -> attn@v MMs, interleaved so the PE always has
        # attn work for chunk tkc while ACT computes exp for chunk tkc+1.
        with (
            tc.tile_pool(name="ps_sc", bufs=2, space="PSUM") as ps_sc,
            tc.tile_pool(name="ps_o", bufs=2, space="PSUM") as ps_o,
        ):
            # Deferred-work queue: each block's normalization chain and the
            # per-batch output-projection chunks are emitted one-per-tkc
            # inside LATER blocks' loops, so the scores/exp/attn drum never
            # pauses.
            pending = []
            _state = {"drain": False}

            def norm_item(b, tqc, h, ou2, rs):
                # ou2: stacked unnormalized attn out [128, TQC] bf16 (SBUF);
                #      this head's rows live at partitions h*64..h*64+63
                # rs: softmax denominators [1, TQC] f32 (SBUF)
                def emit():
                    rc = small.tile([1, TQC], F32, tag="recip", bufs=2,
                                    name=f"rc_{b}_{tqc}_{h}")
                    nc.vector.reciprocal_approx_fast(out=rc, in_=rs)
                    rbs = small.tile([128, TQC], F32, tag="recipb", bufs=2,
                                      name=f"rbs_{b}_{tqc}_{h}")
                    nc.gpsimd.partition_broadcast(
                        out_ap=rbs, in_ap=rc, channels=128)
                    hsl = slice(h * HD, (h + 1) * HD)
                    nc.vector.tensor_mul(
                        on_sb[hsl, b, tqc * TQC:(tqc + 1) * TQC],
                        ou2[hsl, :], rbs[hsl, :])
                    if h == HPC - 1:
                        # both heads of (b, tqc) normalized -> the matching
                        # output-projection chunks are now eligible
                        for ci in range(tqc * 8, tqc * 8 + 8):
                            pending.append(y_item(b, ci))
                return emit

            def y_item(b, ci):
                def emit():
                    psy = ps_sc.tile([128, D], F32, tag="ps_sc",
                                     name=f"psy_{b}_{ci}")
                    for eo in range(D // 512):
                        nc.tensor.matmul(
                            psy[:, eo * 512:(eo + 1) * 512],
                            on_sb[:, b, ci * 128:(ci + 1) * 128],
                            wo_sb[:, eo * 512:(eo + 1) * 512],
                            start=True, stop=True)
                    ysb = small.tile([128, D], BF, tag="ysb", bufs=4,
                                     name=f"ysb_{b}_{ci}")
                    if _state["drain"] and ci % 2 == 1:
                        nc.scalar.copy(ysb, psy)
                    else:
                        nc.vector.tensor_copy(ysb, psy)
                    nc.sync.dma_start(
                        out=y[b * N + ci * 128: b * N + (ci + 1) * 128, :],
                        in_=ysb)
                return emit

            for b in range(B):
                for tqc in range(NTQC):
                    tq0 = b * N + tqc * TQC  # global tq base
                    exp_t = xbig.tile([128, HPC, NKC, TQC], BF, tag="big")
                    ots = [ps_o.tile([HD + 1, TQC], F32, tag="ps_o",
                                     name=f"ot_{b}_{tqc}_{h}")
                           for h in range(HPC)]

                    def attn_mms(j):
                        # attn@v for key chunk j (consumes exp_t[:, :, j, :])
                        for h in range(HPC):
                            vt = v_sb[:, b * NKC + j, h, :]
                            for nn in range(TQC // 512):
                                nc.tensor.matmul(
                                    ots[h][:, nn * 512:(nn + 1) * 512], vt,
                                    exp_t[:, h, j, nn * 512:(nn + 1) * 512],
                                    start=(j == 0), stop=(j == NKC - 1))

                    # Software-pipelined: the PE's attn@v for chunk tkc-1 is
                    # emitted after exp(tkc) so the PE never waits on the
                    # same-iteration exp; ACT (exp) is the steady-state drum.
                    # One deferred item (prev block's normalization / y-proj
                    # chunk) is woven in per tkc iteration.
                    for tkc in range(NKC):
                        scs = []
                        for h in range(HPC):
                            sc = ps_sc.tile([128, TQC], F32, tag="ps_sc")
                            lhsT = k2[h * HD:(h + 1) * HD,
                                      b * N + tkc * 128: b * N + (tkc + 1) * 128]
                            for nn in range(TQC // 512):
                                nc.tensor.matmul(
                                    sc[:, nn * 512:(nn + 1) * 512], lhsT,
                                    q2[h * HD:(h + 1) * HD,
                                       tq0 + nn * 512: tq0 + (nn + 1) * 512],
                                    start=True, stop=True)
                            scs.append(sc)
                        for h in range(HPC):
                            nc.scalar.activation(
                                out=exp_t[:, h, tkc, :], in_=scs[h],
                                func=mybir.ActivationFunctionType.Exp,
                                scale=float(HD) ** -0.5)
                        if tkc > 0:
                            attn_mms(tkc - 1)
                        if tkc >= 1 and pending:
                            pending.pop(0)()
                    attn_mms(NKC - 1)

                    # Evacuate the ot PSUM tiles quickly so the next block's
                    # attn matmuls get the slots; the reciprocal chain is
                    # deferred via `pending`.
                    # Each head: psum rows 0..63 = out, row 64 = denom.
                    ou2 = small.tile([128, TQC], BF, tag="ou", bufs=2,
                                     name=f"ou_{b}_{tqc}")
                    nc.vector.tensor_copy(ou2[0:HD, :], ots[0][0:HD, :])
                    nc.scalar.copy(ou2[HD:2 * HD, :], ots[1][0:HD, :])
                    rss = []
                    for h in range(HPC):
                        rs = small.tile([1, TQC], F32, tag="rs", bufs=4,
                                        name=f"rs_{b}_{tqc}_{h}")
                        if h == 0:
                            nc.vector.tensor_copy(rs, ots[h][HD:HD + 1, :])
                        else:
                            nc.scalar.copy(rs, ots[h][HD:HD + 1, :])
                        rss.append(rs)
                    for h in range(HPC):
                        pending.append(norm_item(b, tqc, h, ou2, rss[h]))

            # drain remaining deferred work (last block's norms + final ys)
            _state["drain"] = True
            while pending:
                pending.pop(0)()


def _host_inputs(x, Wq, Wk, Wv, Wo, bq, bk, bv, bo):
    """Build the 8 per-core input maps (host-side sharding + layout prep)."""
    bf16 = ml_dtypes.bfloat16
    xTh = np.ascontiguousarray(x.reshape(T, D).T).astype(bf16)

    # rope tables: row e uses freq (e % 64) % 32; positions along columns
    i = (np.arange(E) % HD) % (HD // 2)
    inv_freq = ROPE_BASE ** (-2.0 * i / HD)  # [E]
    ang = np.arange(N)[None, :] * inv_freq[:, None]          # [E, N]
    cosb = np.cos(ang).astype(bf16)
    sinb = np.sin(ang).astype(bf16)

    # rotate-half permutation: rot = P @ q (per 64-block)
    P = np.zeros((E, E), dtype=np.float32)
    for h in range(HPC):
        for j in range(HD // 2):
            P[h * HD + j, h * HD + j + HD // 2] = -1.0
            P[h * HD + j + HD // 2, h * HD + j] = 1.0
    rotT = np.ascontiguousarray(P.T).astype(bf16)
    idT = np.eye(E, dtype=np.float32).astype(bf16)

    ones1 = np.ones((1, TQC), dtype=np.float32)

    in_maps = []
    for c in range(NCORES):
        sl = slice(c * E, (c + 1) * E)
        in_maps.append({
            "xT": xTh,
            "wqT": np.ascontiguousarray(Wq[sl, :].T).astype(bf16),
            "wkT": np.ascontiguousarray(Wk[sl, :].T).astype(bf16),
            "wvT": np.ascontiguousarray(Wv[sl, :].T).astype(bf16),
            "woT": np.ascontiguousarray(Wo[:, sl].T).astype(bf16),
            "bcol": np.stack([bq[sl], bk[sl], bv[sl]], axis=1)
                .astype(np.float32),
            "cosb": cosb,
            "sinb": sinb,
            "rotT": rotT,
            "idT": idT,
            "ones1": ones1,
        })
    return in_maps


_NC = None


def kernel(x, Wq, Wk, Wv, Wo, bq, bk, bv, bo):
    from concourse.bass_utils import run_bass_kernel_spmd

    global _NC
    if _NC is None:
        _NC = build_nc()
    bo = np.asarray(bo, dtype=np.float32)
    in_maps = _host_inputs(np.asarray(x, dtype=np.float32),
                           np.asarray(Wq, dtype=np.float32),
                           np.asarray(Wk, dtype=np.float32),
                           np.asarray(Wv, dtype=np.float32),
                           np.asarray(Wo, dtype=np.float32),
                           np.asarray(bq, dtype=np.float32),
                           np.asarray(bk, dtype=np.float32),
                           np.asarray(bv, dtype=np.float32),
                           bo)
    res = run_bass_kernel_spmd(_NC, in_maps, core_ids=list(range(NCORES)))
    out = np.zeros((T, D), dtype=np.float32)
    for r in res.results:
        out += np.asarray(r["y"], dtype=np.float32)
    out += bo[None, :]
    return out.reshape(B, N, D)
